# revision 43
# baseline (speedup 1.0000x reference)
"""Spiking NN (snntorch Leaky x2 + linear readout) on 8 TRN2 NeuronCores.

Data-parallel over batch (32 rows/core). Key restructuring: cur1 is
constant across the 25 timesteps, so the layer-1 membrane scan can run
up front, producing ALL spike inputs spk1[t] for layer 2. The 25
per-step [B,4096]x[4096,4096] matmuls then batch into one matmul with
800 moving columns (25 steps x 32 batch) per core. The layer-2 scan and
the W3 readout accumulate per 128-row output block as it is produced,
so spk2 never needs to be stored whole.

Per-step dynamics (reset == previous spike since THR=1):
  mem = beta*mem + cur - spk_prev;  spk = (mem > 1)
"""
import os
import sys

_TRN_REPO = "/opt/trn_rl_repo"
if _TRN_REPO not in sys.path:
    sys.path.insert(0, _TRN_REPO)

import numpy as np
import ml_dtypes

import concourse.bass as bass
import concourse.bacc as bacc
import concourse.mybir as mybir
from concourse.tile import TileContext
from concourse.bass_utils import run_bass_kernel_spmd

B = 256
NI = 784
NIP = 896          # 784 padded to 7*128
H = 4096
O = 10
T = 25
BETA = 0.95
THR = 1.0
NCORES = 8
BC = B // NCORES   # 32 batch rows per core
CPK = T * BC       # 800 time-batch columns per core
NKH = H // 128     # 32
NKI = NIP // 128   # 7
CH = [(0, 448), (448, 352)]   # psum column chunks (<=512 fp32)
GJ = 4             # h-tile group size for the layer-2 scan
NG = NKH // GJ     # 8

STRATEGY = os.environ.get("SNN_STRATEGY", "fp8mix")  # fp8mix|bf16xN|fp16xN|f32
TRACE = os.environ.get("SNN_TRACE", "0") == "1"
LAST_EXEC_NS = None

F32 = mybir.dt.float32
F32R = mybir.dt.float32r
BF16 = mybir.dt.bfloat16
FP16 = mybir.dt.float16
FP8 = mybir.dt.float8e4
AF = mybir.ActivationFunctionType
OP = mybir.AluOpType
LO_SHIFT = 24   # fp8 residual split is pre-scaled by 2^24


def _build(strategy):
    nc = bacc.Bacc()

    batch_t = nc.dram_tensor("batchT", [NIP, BC], F32, kind="ExternalInput")
    w1 = nc.dram_tensor("w1", [NIP, H], F32, kind="ExternalInput")
    b2t = nc.dram_tensor("b2t", [128, NKH], F32, kind="ExternalInput")
    b3c = nc.dram_tensor("b3c", [O, 1], F32, kind="ExternalInput")
    idn_d = nc.dram_tensor("idn", [BC, BC], F32, kind="ExternalInput")
    fp8lo = strategy == "fp8mix"
    if fp8lo:
        # hi split fp16 (1 col/cycle) + fp8e4 residual scaled by 2^LO_SHIFT
        # streamed DoubleRow (2 k-tiles per pass = 0.5 col/cycle)
        nsp = 2            # w3 readout stays fp16 x2
        mmdt = FP16
        w2s = [nc.dram_tensor("w2hi", [NKH, 128, H], FP16,
                              kind="ExternalInput"),
               nc.dram_tensor("w2lo", [NKH, 128, H], FP8,
                              kind="ExternalInput")]
    elif strategy.startswith("bf16x") or strategy.startswith("fp16x"):
        hdt = FP16 if strategy.startswith("fp16x") else BF16
        nsp = int(strategy[-1])
        splits = ("hi", "mid", "lo")[:nsp]
        w2s = [nc.dram_tensor(f"w2{s}", [NKH, 128, H], hdt, kind="ExternalInput")
               for s in splits]
        mmdt = hdt
    else:
        mmdt = F32R if strategy == "f32r" else F32
        nsp = 1
        w2s = [nc.dram_tensor("w2", [NKH, 128, H], mmdt, kind="ExternalInput")]
    # W3 splits packed along columns so one readout matmul covers all
    # splits; each split starts at a 32-partition boundary (engine APs
    # require partition base in {0,32,64,96}), zero-padded between.
    NO = 32 * nsp if nsp > 1 else O
    w3pk = nc.dram_tensor("w3pk", [H, NO], mmdt, kind="ExternalInput")
    out3 = nc.dram_tensor("out3", [O, CPK], F32, kind="ExternalOutput")

    # spike storage dtype in SBUF: matmul rhs dtype (binary values exact
    # in bf16; f32r is bit-identical to f32 so we write f32 and bitcast)
    spk_store = mmdt if (fp8lo or strategy.startswith(("bf16x", "fp16x"))) \
        else F32

    def rmm(ap):
        # view an f32-written AP as the matmul dtype
        if strategy == "f32r":
            return ap.bitcast(F32R)
        return ap

    with TileContext(nc) as tc:
        with (
            tc.tile_pool(name="resident", bufs=1) as rpool,
            tc.tile_pool(name="ppark", bufs=1, space="PSUM") as ppark,
        ):
            bT = rpool.tile([128, NKI * BC], F32)
            nc.sync.dma_start(
                out=bT.rearrange("p (k b) -> p k b", b=BC),
                in_=batch_t.rearrange("(k p) b -> p k b", p=128))
            b2sb = rpool.tile([128, NKH], F32)
            nc.sync.dma_start(out=b2sb, in_=b2t[:, :])
            b3sb = rpool.tile([O, 1], F32)
            nc.sync.dma_start(out=b3sb, in_=b3c[:, :])
            idn = rpool.tile([BC, BC], F32)
            nc.sync.dma_start(out=idn, in_=idn_d[:, :])
            w3sb = rpool.tile([128, NKH * NO], mmdt)
            nc.sync.dma_start(
                out=w3sb.rearrange("p (k o) -> p k o", o=NO),
                in_=w3pk.rearrange("(k p) o -> p k o", p=128))

            cur1 = rpool.tile([128, NKH * BC], F32)
            mem1 = rpool.tile([128, NKH * BC], F32)
            mem2 = rpool.tile([128, NKH * BC], F32)
            spk1 = rpool.tile([128, NKH * CPK], spk_store)
            s3d = spk1.rearrange("p (k c) -> p k c", c=CPK)
            if fp8lo:
                spk1q = rpool.tile([128, NKH * CPK], FP8)
                sq3 = spk1q.rearrange("p (k c) -> p k c", c=CPK)

            po = [ppark.tile([NO, cw], F32, name=f"po{ci}")
                  for ci, (c0, cw) in enumerate(CH)]

            # ---- Phase A: cur1 = relu(x @ W1.T + b1), layer-1 scan ----
            # batch-stationary (lhsT = x k-tile [128,32]) so the PE streams
            # long w1 rows instead of reloading a [128,128] stationary per
            # 32-col matmul; b1 is folded into pad row 784 (x row == 1).
            # PE-transpose ([32,128] -> [128,32]) restores the
            # hidden-on-partition layout that the scans and Phase B need.
            def scan1(g):
                m = mem1[:, g * GJ * BC:(g + 1) * GJ * BC]
                m3 = m.rearrange("p (j b) -> p j b", b=BC)
                c3 = cur1[:, g * GJ * BC:(g + 1) * GJ * BC].rearrange(
                    "p (j b) -> p j b", b=BC)
                sg = s3d[:, g * GJ:(g + 1) * GJ, :]
                nc.vector.tensor_copy(out=m3, in_=c3)
                nc.vector.tensor_scalar(
                    out=sg[:, :, 0:BC], in0=m3, scalar1=THR, scalar2=None,
                    op0=OP.is_gt)
                for t in range(1, T):
                    nc.vector.scalar_tensor_tensor(
                        out=m3, in0=m3, scalar=BETA, in1=c3,
                        op0=OP.mult, op1=OP.add)
                    nc.vector.tensor_tensor(
                        out=m3, in0=m3, in1=sg[:, :, (t - 1) * BC:t * BC],
                        op=OP.subtract)
                    nc.vector.tensor_scalar(
                        out=sg[:, :, t * BC:(t + 1) * BC], in0=m3,
                        scalar1=THR, scalar2=None, op0=OP.is_gt)

            CW1 = GJ * 128            # 512 hidden cols per chunk == scan group
            with (
                tc.tile_pool(name="w1p", bufs=2) as w1p,
                tc.tile_pool(name="t1p", bufs=2) as t1p,
                tc.tile_pool(name="ps1", bufs=2, space="PSUM") as ps1,
                tc.tile_pool(name="pt1", bufs=3, space="PSUM") as pt1,
            ):
                w1r = w1.rearrange("(k p) m -> p k m", p=128)
                tmp_of = {}

                def transpose_group(g):
                    tmp = tmp_of.pop(g)
                    for jj in range(GJ):
                        j1 = g * GJ + jj
                        pt = pt1.tile([128, BC], F32)
                        nc.tensor.transpose(
                            pt, tmp[:, jj * 128:(jj + 1) * 128], idn)
                        nc.scalar.copy(
                            out=cur1[:, j1 * BC:(j1 + 1) * BC], in_=pt)
                    scan1(g)
                    if fp8lo:
                        nc.scalar.copy(
                            out=spk1q[:, g * GJ * CPK:(g + 1) * GJ * CPK],
                            in_=spk1[:, g * GJ * CPK:(g + 1) * GJ * CPK])

                for g in range(NG):
                    w1c = w1p.tile([128, NKI * CW1], F32)
                    w1c3 = w1c.rearrange("p (k m) -> p k m", m=CW1)
                    hw = CW1 // 2
                    for k in range(NKI):
                        for s in range(2):
                            nc.sync.dma_start(
                                out=w1c3[:, k:k + 1, s * hw:(s + 1) * hw],
                                in_=w1r[:, k:k + 1,
                                        g * CW1 + s * hw:g * CW1 + (s + 1) * hw])
                    pa = ps1.tile([BC, CW1], F32)
                    for k in range(NKI):
                        nc.tensor.matmul(
                            pa, lhsT=bT[:, k * BC:(k + 1) * BC],
                            rhs=w1c3[:, k:k + 1, :],
                            start=(k == 0), stop=(k == NKI - 1))
                    tmp = t1p.tile([BC, CW1], F32)
                    nc.scalar.activation(
                        out=tmp, in_=pa, func=AF.Relu, scale=1.0)
                    tmp_of[g] = tmp
                    if g >= 1:
                        transpose_group(g - 1)
                transpose_group(NG - 1)

            # ---- Phase B: big matmul + layer-2 scan + readout ----
            def scan2(g, ct, sp2):
                m = mem2[:, g * GJ * BC:(g + 1) * GJ * BC]
                m3 = m.rearrange("p (j b) -> p j b", b=BC)
                c3 = ct.rearrange("p (j c) -> p j c", c=CPK)
                sp3 = sp2.rearrange("p (j c) -> p j c", c=CPK)
                nc.vector.tensor_copy(out=m3, in_=c3[:, :, 0:BC])
                nc.vector.tensor_scalar(
                    out=sp3[:, :, 0:BC], in0=m3, scalar1=THR, scalar2=None,
                    op0=OP.is_gt)
                for t in range(1, T):
                    nc.vector.scalar_tensor_tensor(
                        out=m3, in0=m3, scalar=BETA,
                        in1=c3[:, :, t * BC:(t + 1) * BC],
                        op0=OP.mult, op1=OP.add)
                    nc.vector.tensor_tensor(
                        out=m3, in0=m3, in1=sp3[:, :, (t - 1) * BC:t * BC],
                        op=OP.subtract)
                    nc.vector.tensor_scalar(
                        out=sp3[:, :, t * BC:(t + 1) * BC], in0=m3,
                        scalar1=THR, scalar2=None, op0=OP.is_gt)

            def out3_mms(g, sp2):
                for jj in range(GJ):
                    j = g * GJ + jj
                    for ci, (c0, cw) in enumerate(CH):
                        nc.tensor.matmul(
                            po[ci],
                            lhsT=rmm(w3sb[:, j * NO:(j + 1) * NO]),
                            rhs=rmm(sp2[:, jj * CPK + c0:
                                        jj * CPK + c0 + cw]),
                            start=(j == 0),
                            stop=(j == NKH - 1),
                            skip_group_check=True)

            with (
                tc.tile_pool(name="w2p", bufs=2) as w2p,
                tc.tile_pool(name="ctp", bufs=3) as ctp,
                tc.tile_pool(name="sp2p", bufs=3) as sp2p,
                tc.tile_pool(name="ps2", bufs=(3 if fp8lo else 4),
                             space="PSUM") as ps2,
                tc.tile_pool(name="pslo", bufs=(3 if fp8lo else 1),
                             space="PSUM") as pslo,
            ):
                gdata = {}
                for g in range(NG):
                    ct = ctp.tile([128, GJ * CPK], F32)
                    if fp8lo or strategy.startswith(("bf16x", "fp16x")):
                        sp2 = sp2p.tile([128, GJ * CPK], spk_store)
                    else:
                        sp2 = ct  # spikes overwrite cur2 in place
                    for jj in range(GJ):
                        j = g * GJ + jj
                        slabs = []
                        for wi, w2d in enumerate(w2s):
                            slab = w2p.tile([128, H], w2d.dtype,
                                            name=f"sl{wi}")
                            nst = 4  # column strips per slab DMA
                            for si in range(nst):
                                c = H // nst
                                nc.sync.dma_start(
                                    out=slab[:, si * c:(si + 1) * c],
                                    in_=w2d[j, :, si * c:(si + 1) * c])
                            slabs.append(slab)
                        for ci, (c0, cw) in enumerate(CH):
                            ps = ps2.tile([128, cw], F32)
                            tsl = ct[:, jj * CPK + c0:jj * CPK + c0 + cw]
                            if fp8lo:
                                for k in range(NKH):
                                    nc.tensor.matmul(
                                        ps,
                                        lhsT=slabs[0][:, k * 128:(k + 1) * 128],
                                        rhs=spk1[:, k * CPK + c0:
                                                 k * CPK + c0 + cw],
                                        start=(k == 0), stop=(k == NKH - 1))
                                pl = pslo.tile([128, cw], F32)
                                for k2 in range(NKH // 2):
                                    nc.tensor.matmul(
                                        pl,
                                        lhsT=slabs[1][:, 2 * k2 * 128:
                                                      (2 * k2 + 2) * 128]
                                        .rearrange("p (i m) -> p i m", i=2),
                                        rhs=sq3[:, 2 * k2:2 * k2 + 2,
                                                c0:c0 + cw],
                                        start=(k2 == 0),
                                        stop=(k2 == NKH // 2 - 1),
                                        perf_mode=mybir.MatmulPerfMode.DoubleRow,
                                        skip_group_check=True)
                                nc.scalar.activation(
                                    out=tsl, in_=pl, func=AF.Identity,
                                    scale=2.0 ** -LO_SHIFT)
                                nc.vector.tensor_tensor(
                                    out=tsl, in0=tsl, in1=ps, op=OP.add)
                                nc.scalar.activation(
                                    out=tsl, in_=tsl, func=AF.Relu,
                                    bias=b2sb[:, j:j + 1], scale=1.0)
                                continue
                            n_mm = 0
                            for k in range(NKH):
                                for wi in range(nsp):
                                    nc.tensor.matmul(
                                        ps,
                                        lhsT=slabs[wi][:, k * 128:(k + 1) * 128],
                                        rhs=rmm(spk1[:, k * CPK + c0:
                                                     k * CPK + c0 + cw]),
                                        start=(n_mm == 0),
                                        stop=(n_mm == NKH * nsp - 1))
                                    n_mm += 1
                            nc.scalar.activation(
                                out=tsl,
                                in_=ps, func=AF.Relu,
                                bias=b2sb[:, j:j + 1], scale=1.0)
                    scan2(g, ct, sp2)
                    gdata[g] = sp2
                    if g >= 1:
                        out3_mms(g - 1, gdata[g - 1])
                out3_mms(NG - 1, gdata[NG - 1])

                res = rpool.tile([O, CPK], F32)
                for ci, (c0, cw) in enumerate(CH):
                    if nsp == 1:
                        nc.scalar.activation(
                            out=res[:, c0:c0 + cw], in_=po[ci],
                            func=AF.Identity, bias=b3sb, scale=1.0)
                        continue
                    # tensor_tensor may read at most one PSUM input:
                    # evict hi split (+bias) to SBUF first, then add lo
                    nc.scalar.activation(
                        out=res[:, c0:c0 + cw], in_=po[ci][0:O, :],
                        func=AF.Identity, bias=b3sb, scale=1.0)
                    for si in range(1, nsp):
                        nc.vector.tensor_tensor(
                            out=res[:, c0:c0 + cw], in0=res[:, c0:c0 + cw],
                            in1=po[ci][si * 32:si * 32 + O, :], op=OP.add)
                nc.sync.dma_start(out=out3[:, :], in_=res)

    nc.finalize()
    return nc


def _split(full, hdt, nsp):
    out = []
    rem = full
    for _ in range(nsp):
        s = rem.astype(hdt)
        out.append(s)
        rem = rem - s.astype(np.float32)
    return out


def _host_prep(batch, W1, b1, W2, b2, W3, b3, strategy):
    batch = np.ascontiguousarray(batch, dtype=np.float32)
    w1p = np.zeros((NIP, H), dtype=np.float32)
    w1p[:NI] = np.asarray(W1, dtype=np.float32).T
    w1p[NI] = np.asarray(b1, dtype=np.float32)   # bias row (x row == 1)
    b2t = np.ascontiguousarray(
        np.asarray(b2, dtype=np.float32).reshape(NKH, 128).T)
    b3c = np.ascontiguousarray(
        np.asarray(b3, dtype=np.float32).reshape(O, 1))

    W2 = np.asarray(W2, dtype=np.float32)
    w2p = np.ascontiguousarray(
        W2.reshape(NKH, 128, NKH, 128).transpose(0, 3, 2, 1)
        .reshape(NKH, 128, H))
    w3p = np.ascontiguousarray(np.asarray(W3, dtype=np.float32).T)

    common = {"w1": w1p, "b2t": b2t, "b3c": b3c,
              "idn": np.eye(BC, dtype=np.float32)}
    if strategy == "fp8mix":
        hi = w2p.astype(np.float16)
        common["w2hi"] = hi
        resid = (w2p - hi.astype(np.float32)) * (2.0 ** LO_SHIFT)
        common["w2lo"] = resid.astype(ml_dtypes.float8_e4m3fn)
        w3s = _split(w3p, np.float16, 2)
        w3z = np.zeros((H, 64), dtype=np.float16)
        for si, s in enumerate(w3s):
            w3z[:, si * 32:si * 32 + O] = s
        common["w3pk"] = w3z
    elif strategy.startswith(("bf16x", "fp16x")):
        hdt = np.float16 if strategy.startswith("fp16x") else ml_dtypes.bfloat16
        nsp = int(strategy[-1])
        for sname, sval in zip(("hi", "mid", "lo"),
                               _split(w2p, hdt, nsp)):
            common["w2" + sname] = sval
        w3s = _split(w3p, hdt, nsp)
        w3z = np.zeros((H, 32 * nsp), dtype=hdt)
        for si, s in enumerate(w3s):
            w3z[:, si * 32:si * 32 + O] = s
        common["w3pk"] = w3z
    else:
        common["w2"] = w2p
        common["w3pk"] = w3p.astype(np.float32)

    in_maps = []
    for c in range(NCORES):
        bt = np.zeros((NIP, BC), dtype=np.float32)
        bt[:NI] = batch[c * BC:(c + 1) * BC].T
        bt[NI] = 1.0                              # bias row
        m = dict(common)
        m["batchT"] = bt
        in_maps.append(m)
    return in_maps


def kernel(batch, W1, b1, W2, b2, W3, b3):
    global LAST_EXEC_NS
    nc = _build(STRATEGY)
    in_maps = _host_prep(batch, W1, b1, W2, b2, W3, b3, STRATEGY)
    res = run_bass_kernel_spmd(nc, in_maps, list(range(NCORES)), trace=TRACE)
    LAST_EXEC_NS = getattr(res, "exec_time_ns", None)
    out = np.empty((B, T, O), dtype=np.float32)
    for c in range(NCORES):
        r = np.asarray(res.results[c]["out3"])  # [O, 800]
        out[c * BC:(c + 1) * BC] = r.reshape(O, T, BC).transpose(2, 1, 0)
    return out


# revision 44
# speedup vs baseline: 1.0053x; 1.0053x over previous
"""Spiking NN (snntorch Leaky x2 + linear readout) on 8 TRN2 NeuronCores.

Data-parallel over batch (32 rows/core). Key restructuring: cur1 is
constant across the 25 timesteps, so the layer-1 membrane scan can run
up front, producing ALL spike inputs spk1[t] for layer 2. The 25
per-step [B,4096]x[4096,4096] matmuls then batch into one matmul with
800 moving columns (25 steps x 32 batch) per core. The layer-2 scan and
the W3 readout accumulate per 128-row output block as it is produced,
so spk2 never needs to be stored whole.

Per-step dynamics (reset == previous spike since THR=1):
  mem = beta*mem + cur - spk_prev;  spk = (mem > 1)
"""
import os
import sys

_TRN_REPO = "/opt/trn_rl_repo"
if _TRN_REPO not in sys.path:
    sys.path.insert(0, _TRN_REPO)

import numpy as np
import ml_dtypes

import concourse.bass as bass
import concourse.bacc as bacc
import concourse.mybir as mybir
from concourse.tile import TileContext
from concourse.bass_utils import run_bass_kernel_spmd

B = 256
NI = 784
NIP = 896          # 784 padded to 7*128
H = 4096
O = 10
T = 25
BETA = 0.95
THR = 1.0
NCORES = 8
BC = B // NCORES   # 32 batch rows per core
CPK = T * BC       # 800 time-batch columns per core
NKH = H // 128     # 32
NKI = NIP // 128   # 7
CH = [(0, 448), (448, 352)]   # psum column chunks (<=512 fp32)
GJ = 4             # h-tile group size for the layer-2 scan
NG = NKH // GJ     # 8

STRATEGY = os.environ.get("SNN_STRATEGY", "fp8mix")  # fp8mix|bf16xN|fp16xN|f32
TRACE = os.environ.get("SNN_TRACE", "0") == "1"
LAST_EXEC_NS = None

F32 = mybir.dt.float32
F32R = mybir.dt.float32r
BF16 = mybir.dt.bfloat16
FP16 = mybir.dt.float16
FP8 = mybir.dt.float8e4
AF = mybir.ActivationFunctionType
OP = mybir.AluOpType
LO_SHIFT = 24   # fp8 residual split is pre-scaled by 2^24


def _build(strategy):
    nc = bacc.Bacc()

    batch_t = nc.dram_tensor("batchT", [NIP, BC], F32, kind="ExternalInput")
    w1 = nc.dram_tensor("w1", [NIP, H], F32, kind="ExternalInput")
    b2t = nc.dram_tensor("b2t", [128, NKH], F32, kind="ExternalInput")
    b3c = nc.dram_tensor("b3c", [O, 1], F32, kind="ExternalInput")
    idn_d = nc.dram_tensor("idn", [BC, BC], F32, kind="ExternalInput")
    fp8lo = strategy == "fp8mix"
    if fp8lo:
        # hi split fp16 (1 col/cycle) + fp8e4 residual scaled by 2^LO_SHIFT
        # streamed DoubleRow (2 k-tiles per pass = 0.5 col/cycle)
        nsp = 2            # w3 readout stays fp16 x2
        mmdt = FP16
        w2s = [nc.dram_tensor("w2hi", [NKH, 128, H], FP16,
                              kind="ExternalInput"),
               nc.dram_tensor("w2lo", [NKH, 128, H], FP8,
                              kind="ExternalInput")]
    elif strategy.startswith("bf16x") or strategy.startswith("fp16x"):
        hdt = FP16 if strategy.startswith("fp16x") else BF16
        nsp = int(strategy[-1])
        splits = ("hi", "mid", "lo")[:nsp]
        w2s = [nc.dram_tensor(f"w2{s}", [NKH, 128, H], hdt, kind="ExternalInput")
               for s in splits]
        mmdt = hdt
    else:
        mmdt = F32R if strategy == "f32r" else F32
        nsp = 1
        w2s = [nc.dram_tensor("w2", [NKH, 128, H], mmdt, kind="ExternalInput")]
    # W3 splits packed along columns so one readout matmul covers all
    # splits; each split starts at a 32-partition boundary (engine APs
    # require partition base in {0,32,64,96}), zero-padded between.
    NO = 32 * nsp if nsp > 1 else O
    w3pk = nc.dram_tensor("w3pk", [H, NO], mmdt, kind="ExternalInput")
    out3 = nc.dram_tensor("out3", [O, CPK], F32, kind="ExternalOutput")

    # spike storage dtype in SBUF: matmul rhs dtype (binary values exact
    # in bf16; f32r is bit-identical to f32 so we write f32 and bitcast)
    spk_store = mmdt if (fp8lo or strategy.startswith(("bf16x", "fp16x"))) \
        else F32

    def rmm(ap):
        # view an f32-written AP as the matmul dtype
        if strategy == "f32r":
            return ap.bitcast(F32R)
        return ap

    with TileContext(nc) as tc:
        with (
            tc.tile_pool(name="resident", bufs=1) as rpool,
            tc.tile_pool(name="ppark", bufs=1, space="PSUM") as ppark,
        ):
            bT = rpool.tile([128, NKI * BC], F32)
            nc.sync.dma_start(
                out=bT.rearrange("p (k b) -> p k b", b=BC),
                in_=batch_t.rearrange("(k p) b -> p k b", p=128))
            b2sb = rpool.tile([128, NKH], F32)
            nc.sync.dma_start(out=b2sb, in_=b2t[:, :])
            b3sb = rpool.tile([O, 1], F32)
            nc.sync.dma_start(out=b3sb, in_=b3c[:, :])
            idn = rpool.tile([BC, BC], F32)
            nc.sync.dma_start(out=idn, in_=idn_d[:, :])
            w3sb = rpool.tile([128, NKH * NO], mmdt)
            nc.sync.dma_start(
                out=w3sb.rearrange("p (k o) -> p k o", o=NO),
                in_=w3pk.rearrange("(k p) o -> p k o", p=128))

            cur1 = rpool.tile([128, NKH * BC], F32)
            mem1 = rpool.tile([128, NKH * BC], F32)
            mem2 = rpool.tile([128, NKH * BC], F32)
            spk1 = rpool.tile([128, NKH * CPK], spk_store)
            s3d = spk1.rearrange("p (k c) -> p k c", c=CPK)
            if fp8lo:
                spk1q = rpool.tile([128, NKH * CPK], FP8)
                sq3 = spk1q.rearrange("p (k c) -> p k c", c=CPK)

            po = [ppark.tile([NO, cw], F32, name=f"po{ci}")
                  for ci, (c0, cw) in enumerate(CH)]

            # ---- Phase A: cur1 = relu(x @ W1.T + b1), layer-1 scan ----
            # batch-stationary (lhsT = x k-tile [128,32]) so the PE streams
            # long w1 rows instead of reloading a [128,128] stationary per
            # 32-col matmul; b1 is folded into pad row 784 (x row == 1).
            # PE-transpose ([32,128] -> [128,32]) restores the
            # hidden-on-partition layout that the scans and Phase B need.
            def scan1(g):
                m = mem1[:, g * GJ * BC:(g + 1) * GJ * BC]
                m3 = m.rearrange("p (j b) -> p j b", b=BC)
                c3 = cur1[:, g * GJ * BC:(g + 1) * GJ * BC].rearrange(
                    "p (j b) -> p j b", b=BC)
                sg = s3d[:, g * GJ:(g + 1) * GJ, :]
                nc.vector.tensor_copy(out=m3, in_=c3)
                nc.vector.tensor_scalar(
                    out=sg[:, :, 0:BC], in0=m3, scalar1=THR, scalar2=None,
                    op0=OP.is_gt)
                for t in range(1, T):
                    nc.vector.scalar_tensor_tensor(
                        out=m3, in0=m3, scalar=BETA, in1=c3,
                        op0=OP.mult, op1=OP.add)
                    nc.vector.tensor_tensor(
                        out=m3, in0=m3, in1=sg[:, :, (t - 1) * BC:t * BC],
                        op=OP.subtract)
                    nc.vector.tensor_scalar(
                        out=sg[:, :, t * BC:(t + 1) * BC], in0=m3,
                        scalar1=THR, scalar2=None, op0=OP.is_gt)

            CW1 = GJ * 128            # 512 hidden cols per chunk == scan group
            with (
                tc.tile_pool(name="w1p", bufs=2) as w1p,
                tc.tile_pool(name="t1p", bufs=2) as t1p,
                tc.tile_pool(name="ps1", bufs=2, space="PSUM") as ps1,
                tc.tile_pool(name="pt1", bufs=3, space="PSUM") as pt1,
            ):
                w1r = w1.rearrange("(k p) m -> p k m", p=128)
                tmp_of = {}

                def transpose_group(g):
                    tmp = tmp_of.pop(g)
                    for jj in range(GJ):
                        j1 = g * GJ + jj
                        pt = pt1.tile([128, BC], F32)
                        nc.tensor.transpose(
                            pt, tmp[:, jj * 128:(jj + 1) * 128], idn)
                        nc.scalar.copy(
                            out=cur1[:, j1 * BC:(j1 + 1) * BC], in_=pt)
                    scan1(g)
                    if fp8lo:
                        nc.scalar.copy(
                            out=spk1q[:, g * GJ * CPK:(g + 1) * GJ * CPK],
                            in_=spk1[:, g * GJ * CPK:(g + 1) * GJ * CPK])

                for g in range(NG):
                    w1c = w1p.tile([128, NKI * CW1], F32)
                    w1c3 = w1c.rearrange("p (k m) -> p k m", m=CW1)
                    hw = CW1 // 2
                    for k in range(NKI):
                        for s in range(2):
                            nc.sync.dma_start(
                                out=w1c3[:, k:k + 1, s * hw:(s + 1) * hw],
                                in_=w1r[:, k:k + 1,
                                        g * CW1 + s * hw:g * CW1 + (s + 1) * hw])
                    pa = ps1.tile([BC, CW1], F32)
                    for k in range(NKI):
                        nc.tensor.matmul(
                            pa, lhsT=bT[:, k * BC:(k + 1) * BC],
                            rhs=w1c3[:, k:k + 1, :],
                            start=(k == 0), stop=(k == NKI - 1))
                    tmp = t1p.tile([BC, CW1], F32)
                    nc.scalar.activation(
                        out=tmp, in_=pa, func=AF.Relu, scale=1.0)
                    tmp_of[g] = tmp
                    if g >= 1:
                        transpose_group(g - 1)
                transpose_group(NG - 1)

            # ---- Phase B: big matmul + layer-2 scan + readout ----
            def scan2(g, ct, sp2):
                m = mem2[:, g * GJ * BC:(g + 1) * GJ * BC]
                m3 = m.rearrange("p (j b) -> p j b", b=BC)
                c3 = ct.rearrange("p (j c) -> p j c", c=CPK)
                sp3 = sp2.rearrange("p (j c) -> p j c", c=CPK)
                nc.vector.tensor_copy(out=m3, in_=c3[:, :, 0:BC])
                nc.vector.tensor_scalar(
                    out=sp3[:, :, 0:BC], in0=m3, scalar1=THR, scalar2=None,
                    op0=OP.is_gt)
                for t in range(1, T):
                    nc.vector.scalar_tensor_tensor(
                        out=m3, in0=m3, scalar=BETA,
                        in1=c3[:, :, t * BC:(t + 1) * BC],
                        op0=OP.mult, op1=OP.add)
                    nc.vector.tensor_tensor(
                        out=m3, in0=m3, in1=sp3[:, :, (t - 1) * BC:t * BC],
                        op=OP.subtract)
                    nc.vector.tensor_scalar(
                        out=sp3[:, :, t * BC:(t + 1) * BC], in0=m3,
                        scalar1=THR, scalar2=None, op0=OP.is_gt)

            def out3_mms(g, sp2):
                for jj in range(GJ):
                    j = g * GJ + jj
                    for ci, (c0, cw) in enumerate(CH):
                        nc.tensor.matmul(
                            po[ci],
                            lhsT=rmm(w3sb[:, j * NO:(j + 1) * NO]),
                            rhs=rmm(sp2[:, jj * CPK + c0:
                                        jj * CPK + c0 + cw]),
                            start=(j == 0),
                            stop=(j == NKH - 1),
                            skip_group_check=True)

            with (
                tc.tile_pool(name="w2p", bufs=3) as w2p,
                tc.tile_pool(name="ctp", bufs=3) as ctp,
                tc.tile_pool(name="sp2p", bufs=3) as sp2p,
                tc.tile_pool(name="ps2", bufs=(3 if fp8lo else 4),
                             space="PSUM") as ps2,
                tc.tile_pool(name="pslo", bufs=(3 if fp8lo else 1),
                             space="PSUM") as pslo,
            ):
                gdata = {}
                for g in range(NG):
                    ct = ctp.tile([128, GJ * CPK], F32)
                    if fp8lo or strategy.startswith(("bf16x", "fp16x")):
                        sp2 = sp2p.tile([128, GJ * CPK], spk_store)
                    else:
                        sp2 = ct  # spikes overwrite cur2 in place
                    for jj in range(GJ):
                        j = g * GJ + jj
                        slabs = []
                        for wi, w2d in enumerate(w2s):
                            slab = w2p.tile([128, H], w2d.dtype,
                                            name=f"sl{wi}")
                            nst = 4  # column strips per slab DMA
                            for si in range(nst):
                                c = H // nst
                                nc.sync.dma_start(
                                    out=slab[:, si * c:(si + 1) * c],
                                    in_=w2d[j, :, si * c:(si + 1) * c])
                            slabs.append(slab)
                        for ci, (c0, cw) in enumerate(CH):
                            ps = ps2.tile([128, cw], F32)
                            tsl = ct[:, jj * CPK + c0:jj * CPK + c0 + cw]
                            if fp8lo:
                                for k in range(NKH):
                                    nc.tensor.matmul(
                                        ps,
                                        lhsT=slabs[0][:, k * 128:(k + 1) * 128],
                                        rhs=spk1[:, k * CPK + c0:
                                                 k * CPK + c0 + cw],
                                        start=(k == 0), stop=(k == NKH - 1))
                                pl = pslo.tile([128, cw], F32)
                                for k2 in range(NKH // 2):
                                    nc.tensor.matmul(
                                        pl,
                                        lhsT=slabs[1][:, 2 * k2 * 128:
                                                      (2 * k2 + 2) * 128]
                                        .rearrange("p (i m) -> p i m", i=2),
                                        rhs=sq3[:, 2 * k2:2 * k2 + 2,
                                                c0:c0 + cw],
                                        start=(k2 == 0),
                                        stop=(k2 == NKH // 2 - 1),
                                        perf_mode=mybir.MatmulPerfMode.DoubleRow,
                                        skip_group_check=True)
                                nc.scalar.activation(
                                    out=tsl, in_=pl, func=AF.Identity,
                                    scale=2.0 ** -LO_SHIFT)
                                nc.vector.tensor_tensor(
                                    out=tsl, in0=tsl, in1=ps, op=OP.add)
                                nc.scalar.activation(
                                    out=tsl, in_=tsl, func=AF.Relu,
                                    bias=b2sb[:, j:j + 1], scale=1.0)
                                continue
                            n_mm = 0
                            for k in range(NKH):
                                for wi in range(nsp):
                                    nc.tensor.matmul(
                                        ps,
                                        lhsT=slabs[wi][:, k * 128:(k + 1) * 128],
                                        rhs=rmm(spk1[:, k * CPK + c0:
                                                     k * CPK + c0 + cw]),
                                        start=(n_mm == 0),
                                        stop=(n_mm == NKH * nsp - 1))
                                    n_mm += 1
                            nc.scalar.activation(
                                out=tsl,
                                in_=ps, func=AF.Relu,
                                bias=b2sb[:, j:j + 1], scale=1.0)
                    scan2(g, ct, sp2)
                    gdata[g] = sp2
                    if g >= 1:
                        out3_mms(g - 1, gdata[g - 1])
                out3_mms(NG - 1, gdata[NG - 1])

                res = rpool.tile([O, CPK], F32)
                for ci, (c0, cw) in enumerate(CH):
                    if nsp == 1:
                        nc.scalar.activation(
                            out=res[:, c0:c0 + cw], in_=po[ci],
                            func=AF.Identity, bias=b3sb, scale=1.0)
                        continue
                    # tensor_tensor may read at most one PSUM input:
                    # evict hi split (+bias) to SBUF first, then add lo
                    nc.scalar.activation(
                        out=res[:, c0:c0 + cw], in_=po[ci][0:O, :],
                        func=AF.Identity, bias=b3sb, scale=1.0)
                    for si in range(1, nsp):
                        nc.vector.tensor_tensor(
                            out=res[:, c0:c0 + cw], in0=res[:, c0:c0 + cw],
                            in1=po[ci][si * 32:si * 32 + O, :], op=OP.add)
                nc.sync.dma_start(out=out3[:, :], in_=res)

    nc.finalize()
    return nc


def _split(full, hdt, nsp):
    out = []
    rem = full
    for _ in range(nsp):
        s = rem.astype(hdt)
        out.append(s)
        rem = rem - s.astype(np.float32)
    return out


def _host_prep(batch, W1, b1, W2, b2, W3, b3, strategy):
    batch = np.ascontiguousarray(batch, dtype=np.float32)
    w1p = np.zeros((NIP, H), dtype=np.float32)
    w1p[:NI] = np.asarray(W1, dtype=np.float32).T
    w1p[NI] = np.asarray(b1, dtype=np.float32)   # bias row (x row == 1)
    b2t = np.ascontiguousarray(
        np.asarray(b2, dtype=np.float32).reshape(NKH, 128).T)
    b3c = np.ascontiguousarray(
        np.asarray(b3, dtype=np.float32).reshape(O, 1))

    W2 = np.asarray(W2, dtype=np.float32)
    w2p = np.ascontiguousarray(
        W2.reshape(NKH, 128, NKH, 128).transpose(0, 3, 2, 1)
        .reshape(NKH, 128, H))
    w3p = np.ascontiguousarray(np.asarray(W3, dtype=np.float32).T)

    common = {"w1": w1p, "b2t": b2t, "b3c": b3c,
              "idn": np.eye(BC, dtype=np.float32)}
    if strategy == "fp8mix":
        hi = w2p.astype(np.float16)
        common["w2hi"] = hi
        resid = (w2p - hi.astype(np.float32)) * (2.0 ** LO_SHIFT)
        common["w2lo"] = resid.astype(ml_dtypes.float8_e4m3fn)
        w3s = _split(w3p, np.float16, 2)
        w3z = np.zeros((H, 64), dtype=np.float16)
        for si, s in enumerate(w3s):
            w3z[:, si * 32:si * 32 + O] = s
        common["w3pk"] = w3z
    elif strategy.startswith(("bf16x", "fp16x")):
        hdt = np.float16 if strategy.startswith("fp16x") else ml_dtypes.bfloat16
        nsp = int(strategy[-1])
        for sname, sval in zip(("hi", "mid", "lo"),
                               _split(w2p, hdt, nsp)):
            common["w2" + sname] = sval
        w3s = _split(w3p, hdt, nsp)
        w3z = np.zeros((H, 32 * nsp), dtype=hdt)
        for si, s in enumerate(w3s):
            w3z[:, si * 32:si * 32 + O] = s
        common["w3pk"] = w3z
    else:
        common["w2"] = w2p
        common["w3pk"] = w3p.astype(np.float32)

    in_maps = []
    for c in range(NCORES):
        bt = np.zeros((NIP, BC), dtype=np.float32)
        bt[:NI] = batch[c * BC:(c + 1) * BC].T
        bt[NI] = 1.0                              # bias row
        m = dict(common)
        m["batchT"] = bt
        in_maps.append(m)
    return in_maps


def kernel(batch, W1, b1, W2, b2, W3, b3):
    global LAST_EXEC_NS
    nc = _build(STRATEGY)
    in_maps = _host_prep(batch, W1, b1, W2, b2, W3, b3, STRATEGY)
    res = run_bass_kernel_spmd(nc, in_maps, list(range(NCORES)), trace=TRACE)
    LAST_EXEC_NS = getattr(res, "exec_time_ns", None)
    out = np.empty((B, T, O), dtype=np.float32)
    for c in range(NCORES):
        r = np.asarray(res.results[c]["out3"])  # [O, 800]
        out[c * BC:(c + 1) * BC] = r.reshape(O, T, BC).transpose(2, 1, 0)
    return out
